# revision 27
# baseline (speedup 1.0000x reference)
"""Cross-attention Trainium2 kernel (8 NeuronCores), v3.

Sharding: core i handles batch b = i//2 and head-group g = i%2 (8 of 16
heads). The host pre-transposes x,c (feature-major), pre-casts all inputs
to bf16, precomputes signed sin/cos RoPE tables, and sums the two
head-group partials per batch (+bias) on the way out.

Device pipeline per core (bf16 matmuls, fp32 accumulation):
  1. inputs DMA'd in consumption order (the DMA engines are a serial
     resource: chunk order == schedule)
  2. Q = x@Wq (natural layout), RoPE on DVE, SBUF->SBUF DMA-transpose
     -> qT (head dims on partitions); same for K -> kT; V natural into
     v_aug with an appended all-ones block column so the AV matmul also
     produces replicated softmax denominators
  3. attention per head over k-blocks: scores^T = kT.T@qT (PSUM
     double-buffered), exp on ScalarE (scale=1/8 fused, bf16 out), AV
     accumulation; K/V-proj blocks and the first half of the output
     projection interleave between attention units as PE filler while
     ScalarE works through the exps
  4. reciprocal of the ones-rows normalizes into A^T; Y = A@Wproj_g in
     two half-contraction passes (partial kept in bf16 SBUF)
"""
import sys

sys.path.insert(0, "/opt/trn_rl_repo")

import numpy as np
import ml_dtypes

import concourse.bass as bass
import concourse.mybir as mybir
from concourse import bacc
from concourse.tile import TileContext
from concourse.bass_utils import run_bass_kernel_spmd

F32 = mybir.dt.float32
BF16 = mybir.dt.bfloat16
AF = mybir.ActivationFunctionType
ALU = mybir.AluOpType

P = 128
DIM = 1024
H = 8          # heads per core
HD = 64        # head dim
QC = 512       # q/k/v columns per core (H*HD)
NX = 1024      # query tokens
NC = 2048      # context tokens
XB = NX // P   # 8 x token blocks
CB = NC // P   # 16 c token blocks
KC = DIM // P  # 8 contraction chunks for projections
MT = QC // P   # 4 head-dim tiles (pairs of heads)


def _emit_rope(nc, pool, psum, trig_cos, trig_sin, tb, out_bf):
    """RoPE on a [128, QC] psum tile viewed as [128, H, HD]; writes bf16.

    trig tables are [128, nblk, HD] bf16 with the rotation sign folded
    into sin (host-precomputed): out = p*cos + rot(p)*sin_signed."""
    pq = psum.rearrange("p (h d) -> p h d", h=H)
    cosb = trig_cos[:, tb, None, :].to_broadcast((P, H, HD))
    sin_lo = trig_sin[:, tb, None, 0:32].to_broadcast((P, H, 32))
    sin_hi = trig_sin[:, tb, None, 32:64].to_broadcast((P, H, 32))
    tmp = pool.tile([P, H, HD], F32, name="rope_tmp", tag="rope_tmp")
    nc.vector.tensor_tensor(tmp[:, :, 0:32], pq[:, :, 32:64], sin_lo, ALU.mult)
    nc.vector.tensor_tensor(tmp[:, :, 32:64], pq[:, :, 0:32], sin_hi, ALU.mult)
    cq = pool.tile([P, H, HD], F32, name="rope_cq", tag="rope_cq")
    nc.vector.tensor_tensor(cq, pq, cosb, ALU.mult)
    ov = out_bf.rearrange("p (h d) -> p h d", h=H)
    nc.vector.tensor_tensor(ov, cq, tmp, ALU.add)


def build_kernel():
    nc = bacc.Bacc("TRN2", target_bir_lowering=False, debug=False)

    xt_d = nc.dram_tensor("xt", [DIM, NX], BF16, kind="ExternalInput")
    ct_d = nc.dram_tensor("ct", [DIM, NC], BF16, kind="ExternalInput")
    wq_d = nc.dram_tensor("wq", [DIM, QC], BF16, kind="ExternalInput")
    wkv_d = nc.dram_tensor("wkv", [DIM, 2 * QC], BF16, kind="ExternalInput")
    wp_d = nc.dram_tensor("wp", [QC, DIM], BF16, kind="ExternalInput")
    trig_d = nc.dram_tensor("trig", [P, (2 * XB + 2 * CB) * HD], BF16,
                            kind="ExternalInput")
    y_d = nc.dram_tensor("y", [NX, DIM], BF16, kind="ExternalOutput")

    with TileContext(nc) as tc:
        with tc.tile_pool(name="persist", bufs=1) as pers, \
             tc.tile_pool(name="stage", bufs=2) as stage, \
             tc.tile_pool(name="ps_proj", bufs=2, space="PSUM") as ps_proj, \
             tc.tile_pool(name="ps_s", bufs=2, space="PSUM") as ps_s, \
             tc.tile_pool(name="ps_av", bufs=1, space="PSUM") as ps_av:

            # ------------- persistent SBUF tensors -------------
            cT = pers.tile([P, KC, NC], BF16)
            wq_sb = pers.tile([P, KC, QC], BF16)
            wkv_sb = pers.tile([P, KC, 2 * QC], BF16)
            wp_sb = pers.tile([P, MT, DIM], BF16)
            trig_sb = pers.tile([P, 2 * XB + 2 * CB, HD], BF16)
            cosx_sb = trig_sb[:, 0:XB, :]
            sinx_sb = trig_sb[:, XB:2 * XB, :]
            cosc_sb = trig_sb[:, 2 * XB:2 * XB + CB, :]
            sinc_sb = trig_sb[:, 2 * XB + CB:2 * XB + 2 * CB, :]
            qT = pers.tile([P, MT, NX], BF16)
            kT = pers.tile([P, MT, NC], BF16)
            v_aug = pers.tile([P, CB, H, P], BF16)  # [...,0:64]=V, 64:128=ones
            a_T = pers.tile([P, MT, NX], BF16)

            wq_v = wq_d.rearrange("(o p) n -> p o n", p=P)
            xt_v = xt_d.rearrange("(o p) n -> p o n", p=P)
            ct_v = ct_d.rearrange("(o p) n -> p o n", p=P)

            with tc.tile_pool(name="xpool", bufs=1) as xpool:
                xT = xpool.tile([P, KC, NX], BF16)

                # ---- input DMAs ----
                # HWDGE (Act queue): Q-side + K/V weights, consumption order.
                # SWDGE (Pool queue): cT chunks + wp, paced by explicit ready
                # times -- its completion-sem window is separate from the
                # HWDGE one that the rope-paced transposes ride on.
                nc.scalar.dma_start(wq_sb, wq_v)
                for xh in range(2):
                    ks = slice(xh * 4, xh * 4 + 4)
                    nc.scalar.dma_start(xT[:, ks, :], xt_v[:, ks, :])
                nc.scalar.dma_start(
                    trig_sb, trig_d.rearrange("p (o d) -> p o d", d=HD))
                nc.scalar.dma_start(wkv_sb,
                                    wkv_d.rearrange("(o p) n -> p o n", p=P))
                nc.gpsimd.memset(v_aug[:, :, :, HD:P], 1.0)
                for ch in range(CB // 2):
                    sl = slice(ch * 2 * P, (ch + 1) * 2 * P)
                    nc.gpsimd.dma_start(cT[:, :, sl], ct_v[:, :, sl])
                nc.gpsimd.dma_start(wp_sb,
                                    wp_d.rearrange("(o p) n -> p o n", p=P))

                # ---- Q projection + RoPE + transpose ----
                # kc-outer over groups of 4 token blocks (PSUM borrowed from
                # the idle scores pool) so each arriving wq/xT chunk feeds
                # 4 blocks' worth of matmuls immediately.
                for g in range(2):
                    pqs = [ps_s.tile([P, NX], F32, name=f"pqg{g}_{i}", tag="s")
                           for i in range(2)]
                    for kc in range(KC):
                        for i in range(4):
                            tb = g * 4 + i
                            nc.tensor.matmul(
                                pqs[i // 2][:, (i % 2) * QC:(i % 2 + 1) * QC],
                                xT[:, kc, tb * P:(tb + 1) * P],
                                wq_sb[:, kc, :],
                                start=(kc == 0), stop=(kc == KC - 1))
                    for i in range(4):
                        tb = g * 4 + i
                        pq = pqs[i // 2][:, (i % 2) * QC:(i % 2 + 1) * QC]
                        q_bf = stage.tile([P, QC], BF16, name=f"qbf{tb}",
                                          tag="q_bf", bufs=3)
                        _emit_rope(nc, stage, pq, cosx_sb, sinx_sb, tb, q_bf)
                        nc.scalar.dma_start_transpose(
                            qT[:, 0:MT, tb * P:(tb + 1) * P], q_bf)

            with tc.tile_pool(name="ypool", bufs=1) as ypool:
                # running Y partial, ping-ponged between passes (bf16)
                yA = ypool.tile([P, XB, DIM], BF16)
                yB = ypool.tile([P, XB, DIM], BF16)

                # ---- K/V projection block ----
                def kv_block(tb):
                    pk = ps_proj.tile([P, QC], F32, name=f"pk{tb}", tag="pp")
                    for kc in range(KC):
                        nc.tensor.matmul(pk, cT[:, kc, tb * P:(tb + 1) * P],
                                         wkv_sb[:, kc, 0:QC],
                                         start=(kc == 0), stop=(kc == KC - 1))
                    k_bf = stage.tile([P, QC], BF16, name=f"kbf{tb}",
                                      tag="q_bf", bufs=3)
                    _emit_rope(nc, stage, pk, cosc_sb, sinc_sb, tb, k_bf)
                    nc.sync.dma_start_transpose(
                        kT[:, 0:MT, tb * P:(tb + 1) * P], k_bf)
                    pv = ps_proj.tile([P, QC], F32, name=f"pv{tb}", tag="pp")
                    for kc in range(KC):
                        nc.tensor.matmul(pv, cT[:, kc, tb * P:(tb + 1) * P],
                                         wkv_sb[:, kc, QC:2 * QC],
                                         start=(kc == 0), stop=(kc == KC - 1))
                    nc.scalar.copy(v_aug[:, tb, :, 0:HD],
                                   pv.rearrange("p (h d) -> p h d", h=H))

                # ---- attention, scores+exp decoupled from AV ----
                # s_part computes scores and exp into a deep e2 FIFO; av_part
                # consumes it. This lets the next head's exps run while the
                # previous head still owns the PSUM accumulator.
                pavs = {}
                e2s = {}

                def s_part(h, m):
                    p, par = h // 2, h % 2
                    rows = slice(par * HD, (par + 1) * HD)
                    s = ps_s.tile([P, NX], F32, name=f"s{h}_{m}", tag="s")
                    for qb in range(2):
                        sl = slice(qb * 512, (qb + 1) * 512)
                        nc.tensor.matmul(s[:, sl],
                                         kT[rows, p, m * P:(m + 1) * P],
                                         qT[rows, p, sl], start=True, stop=True)
                    e2 = stage.tile([P, NX], BF16, name=f"e{h}_{m}", tag="e",
                                    bufs=10)
                    nc.scalar.activation(e2, s, AF.Exp, scale=0.125)
                    e2s[(h, m)] = e2

                def av_part(h, m):
                    p, par = h // 2, h % 2
                    rows = slice(par * HD, (par + 1) * HD)
                    if m == 0:
                        pavs[h] = ps_av.tile([P, NX], F32, name=f"pav{h}",
                                             tag="av")
                    pav = pavs[h]
                    e2 = e2s.pop((h, m))
                    for qb in range(2):
                        sl = slice(qb * 512, (qb + 1) * 512)
                        nc.tensor.matmul(pav[:, sl], v_aug[:, m, h, :],
                                         e2[:, sl],
                                         start=(m == 0), stop=(m == CB - 1))
                    if m == CB - 1:
                        recp = stage.tile([P, NX], F32, name=f"rec{h}",
                                          tag="rec", bufs=1)
                        for half in range(2):
                            sl = slice(half * 512, (half + 1) * 512)
                            nc.vector.reciprocal(recp[64:128, sl],
                                                 pav[64:128, sl])
                            nc.vector.tensor_tensor(a_T[rows, p, sl],
                                                    pav[0:64, sl],
                                                    recp[64:128, sl], ALU.mult)

                # ---- output projection: 4 per-pair passes ----
                # pass p adds a_T[:, p, :] @ wp[p] into the running bf16
                # partial; emitted as PE filler once heads 2p,2p+1 are done.
                def proj_unit(p, i):
                    tb, ob = divmod(i, 2)
                    sl = slice(ob * 512, (ob + 1) * 512)
                    tsl = slice(tb * P, (tb + 1) * P)
                    py = ps_proj.tile([P, 512], F32, name=f"py{p}_{i}",
                                      tag="pp")
                    nc.tensor.matmul(py, a_T[:, p, tsl], wp_sb[:, p, sl],
                                     start=True, stop=True)
                    src_t = yA if p % 2 == 0 else yB
                    dst_t = yB if p % 2 == 0 else yA
                    if p == 0:
                        nc.vector.tensor_copy(yB[:, tb, sl], py)
                    elif p < MT - 1:
                        nc.vector.tensor_tensor(dst_t[:, tb, sl], py,
                                                src_t[:, tb, sl], ALU.add)
                    else:
                        y_sb = stage.tile([P, 512], BF16, name=f"y{tb}_{ob}",
                                          tag="y", bufs=4)
                        if i % 3 == 2:
                            # bounce via Act+Pool to offload the DVE tail
                            t_sb = stage.tile([P, 512], BF16,
                                              name=f"t{tb}_{ob}", tag="tb",
                                              bufs=2)
                            nc.scalar.copy(t_sb, py)
                            nc.gpsimd.tensor_tensor(y_sb, t_sb,
                                                    src_t[:, tb, sl], ALU.add)
                        else:
                            nc.vector.tensor_tensor(y_sb, py,
                                                    src_t[:, tb, sl], ALU.add)
                        nc.sync.dma_start(y_d[tsl, sl], y_sb)

                # ---- interleaved schedule ----
                units = [(h, m) for h in range(H) for m in range(CB)]
                NU = len(units)
                sp = ap = 0      # scores/exp pointer, AV pointer
                pp, pi = 0, 0    # proj filler progress (pass, unit)
                di = 0

                def emit_part(tb):
                    nonlocal sp, ap, pp, pi, di
                    if (sp < NU and sp - ap < 9
                            and (tb is None or units[sp][1] <= tb - 1)):
                        s_part(*units[sp])
                        sp += 1
                    elif ap < sp:
                        h, m = units[ap]
                        av_part(h, m)
                        ap += 1
                        di += 1
                        if di % 2 == 0 and pp < MT - 1 and h > 2 * pp + 1:
                            proj_unit(pp, pi)
                            pi += 1
                            if pi == 16:
                                pp, pi = pp + 1, 0
                        return True
                    else:
                        return False
                    return True

                for tb in range(3):
                    kv_block(tb)
                for tb in range(3, CB):
                    kv_block(tb)
                    for _ in range(6):
                        if not emit_part(tb):
                            break
                while sp < NU or ap < NU:
                    emit_part(None)
                while pp < MT - 1:
                    proj_unit(pp, pi)
                    pi += 1
                    if pi == 16:
                        pp, pi = pp + 1, 0
                for i in range(16):
                    proj_unit(MT - 1, i)
    nc.compile()
    return nc


_NC_CACHE = None
BF = ml_dtypes.bfloat16


def make_in_maps(inputs):
    x, c = inputs["x"], inputs["c"]
    Wq, Wkv, Wproj = inputs["Wq"], inputs["Wkv"], inputs["Wproj"]

    def bft(a):  # bf16 transpose, contiguous
        return np.ascontiguousarray(np.asarray(a, np.float32).T.astype(BF))

    def bf(a):
        return np.ascontiguousarray(np.asarray(a, np.float32).astype(BF))

    thx = np.asarray(inputs["x_pos_embed"], np.float32)
    thc = np.asarray(inputs["c_pos_embed"], np.float32)
    cosx, sinx = np.cos(thx), np.sin(thx).copy()
    cosc, sinc = np.cos(thc), np.sin(thc).copy()
    sinx[:, 0:HD // 2] *= -1.0   # rotation sign folded into the table
    sinc[:, 0:HD // 2] *= -1.0

    xt = [bft(x[b]) for b in range(4)]
    ct = [bft(c[b]) for b in range(4)]
    in_maps = []
    for core in range(8):
        b, g = core // 2, core % 2
        sl = slice(g * QC, (g + 1) * QC)
        wkv = np.concatenate([Wkv[:, sl],
                              Wkv[:, DIM + g * QC: DIM + (g + 1) * QC]],
                             axis=1)
        tcat = np.concatenate([cosx, sinx, cosc, sinc], axis=0)
        # SBUF layout: [p, o, d] where table row = o*128 + p
        trig = np.ascontiguousarray(
            tcat.reshape(48, P, HD).transpose(1, 0, 2).reshape(P, 48 * HD))
        in_maps.append(dict(
            xt=xt[b],
            ct=ct[b],
            wq=bf(Wq[:, sl]),
            wkv=bf(wkv),
            wp=bf(Wproj[sl, :]),
            trig=bf(trig),
        ))
    return in_maps


def kernel(x, c, x_pos_embed, c_pos_embed, Wq, Wkv, Wproj, bproj):
    global _NC_CACHE
    if _NC_CACHE is None:
        _NC_CACHE = build_kernel()
    nc = _NC_CACHE

    B = x.shape[0]
    in_maps = make_in_maps(dict(x=x, c=c, Wq=Wq, Wkv=Wkv, Wproj=Wproj,
                                x_pos_embed=x_pos_embed,
                                c_pos_embed=c_pos_embed))

    res = run_bass_kernel_spmd(nc, in_maps, core_ids=list(range(8)))
    out = np.empty((B, NX, DIM), np.float32)
    bias = np.asarray(bproj, np.float32)
    for b in range(B):
        out[b] = (np.asarray(res.results[2 * b]["y"], np.float32)
                  + np.asarray(res.results[2 * b + 1]["y"], np.float32)
                  + bias)
    return out
